# revision 39
# baseline (speedup 1.0000x reference)
"""Center-loss kernel for Trainium2 (Bass/Bacc, raw), 8-core data-parallel.

loss = 2 - 2 * (sum_i feature[i, label[i]] / 64) / 8192

Sharding: batch dim (8192 rows) split evenly across 8 NeuronCores.
Each core gathers its 1024 picked elements straight out of DRAM with
indirect DMAs (no full-matrix read), reduces them to one scalar partial
sum on-device, and the host combines the 8 partials.

Implementation notes (hard-won on this toolchain):
- Raw Bacc instead of TileContext: Tile's sem-init preamble + tail
  barrier butterfly cost ~16us on a ~4us kernel. Everything here runs
  on the GPSIMD (Pool) engine in-order with ONE DMA semaphore.
- The HW indirect DMA consumes ONE index per dest partition and fetches
  dest-free-size contiguous elements, so gathering 1024 scattered
  elements takes 8 DMAs of [128,1] (index tile column by column).
- labpack packs labels + constant row offsets so the index add is a
  single tensor_tensor with one sem wait (HW: 1 sync wait per inst).
"""

import sys

if "/opt/trn_rl_repo" not in sys.path:
    sys.path.insert(0, "/opt/trn_rl_repo")

import numpy as np

import concourse.bacc as bacc
import concourse.bass as bass
from concourse import mybir
from concourse import bass_utils

N = 8192          # batch rows
C = 10000         # num classes (feature columns)
N_CORES = 8
R = N // N_CORES  # rows per core
P = 128           # SBUF partitions
K = R // P        # picked elements per partition
SCALE = 64.0

_NC_CACHE = None


def _build_nc() -> bacc.Bacc:
    global _NC_CACHE
    if _NC_CACHE is not None:
        return _NC_CACHE

    # Bass.__init__ ends with const-tile memsets + an all-engine barrier;
    # nothing here reads the const tiles, so skip the barrier (~0.8us off
    # the first DMA). The block-exit barrier is emitted later, unpatched.
    _orig_barrier = bass.Bass.all_engine_barrier
    bass.Bass.all_engine_barrier = lambda self, **kw: None
    try:
        nc = bacc.Bacc(trn_type="TRN2", monotonic_sem_count=0)
    finally:
        bass.Bass.all_engine_barrier = _orig_barrier
    feat = nc.dram_tensor("feature", [R, C], mybir.dt.float32, kind="ExternalInput")
    # labpack[0] = labels, labpack[1] = arange(R)*C (constant row offsets).
    lab = nc.dram_tensor("labpack", [2, R], mybir.dt.int32, kind="ExternalInput")
    # Two per-chunk partial sums; the host adds them. (Single-element
    # offset reads — e.g. res2[0:1, 1:2] as a TT operand or a [1, 2]
    # X-reduce — are miscompiled to zero/elem0 on this toolchain, so the
    # final scalar combine stays off-device.)
    out = nc.dram_tensor("out", [1, 3], mybir.dt.float32, kind="ExternalOutput")

    # Chunk boundaries in idx columns (128 picks per column). A small
    # first chunk starts the DMA ring early; a small last chunk keeps
    # the final (critical-path) reduce short.
    CHUNK_COLS = ((0, 1), (1, 7), (7, 8))

    with (
        nc.sbuf_tensor("lp", [P, 2, K], mybir.dt.int32) as lp,
        nc.sbuf_tensor("idx", [P, K], mybir.dt.int32) as idx,
        nc.sbuf_tensor("gat", [1, R], mybir.dt.float32) as gat,
        nc.sbuf_tensor("res2", [1, 3], mybir.dt.float32) as res2,
        nc.sbuf_tensor("warm_idx", [2, 1], mybir.dt.int32) as warm_idx,
        nc.sbuf_tensor("warm_tt", [2, 1], mybir.dt.int32) as warm_tt,
        nc.sbuf_tensor("warm_dst", [1, 2], mybir.dt.float32) as warm_dst,
        nc.semaphore() as S,
        nc.semaphore() as SA,
        nc.semaphore() as SB,
        nc.semaphore() as SC,
        nc.semaphore() as S2,
        nc.Block(no_gpsimd_drain=True) as block,
    ):
        # S: 16 labpack DMA done, 17 idx ready (DVE), 18 reduces done.
        # SA/SB: gather chunk 0/1 done (separate sems — qPoolDynamic chunk
        # completions are NOT ordered). The out DMA only incs S2, a
        # write-only counter nobody waits on (completion is guaranteed by
        # the SP engine's block-exit Drain), so the critical path skips its
        # completion latency and S2 needs no clear. SP clears the waited-on
        # sems once compute is done so re-executions of the loaded NEFF
        # start from zero (NRT only zeroes sems at load).

        @block.sync
        def _(sp):
            # Local row r = p*K + j lives at tile position [p, :, j].
            sp.dma_start(
                out=lp[:], in_=lab[:].rearrange("t (p k) -> p t k", p=P)
            ).then_inc(S, 16)
            sp.wait_ge(S, 18)
            sp.dma_start(out=out[:], in_=res2[:]).then_inc(S2, 16)
            sp.sem_clear(S)
            sp.sem_clear(SA)
            sp.sem_clear(SB)
            sp.sem_clear(SC)

        @block.gpsimd
        def _(g):
            # Warm-up: a 2-descriptor indirect gather at the head of the
            # Pool stream loads the SWDGE desc-gen ucode library while the
            # labpack DMA + index add are still in flight.
            g.memset(warm_idx[:], 0)
            g.indirect_dma_start(
                out=warm_dst[:].rearrange("p (n one) -> p n one", one=1),
                out_offset=None,
                in_=feat[:],
                in_offset=bass.IndirectOffsetOnAxis(ap=warm_idx[:], axis=1),
            ).then_inc(S2, 16)
            g.wait_ge(S, 17)
            # Indirect gathers, chunked so the first chunk's reduce hides
            # under the second chunk's transfer. Walrus generates one
            # descriptor per entry of the dest AP's second-to-last dim, so a
            # [1, H, 1] dest on one partition yields H single-element
            # descriptors; the offset tile is consumed partition-fastest
            # (a permutation of our [p, j] order — irrelevant for the sum).
            for (c0, c1), sem in zip(CHUNK_COLS, (SA, SB, SC)):
                g.indirect_dma_start(
                    out=gat[0:1, c0 * P : c1 * P].rearrange(
                        "p (n one) -> p n one", one=1
                    ),
                    out_offset=None,
                    in_=feat[:],
                    # axis=1 -> coef = prod(shape[2:]) = 1: indices are flat
                    # element offsets into the contiguous [R, C] block.
                    in_offset=bass.IndirectOffsetOnAxis(
                        ap=idx[:, c0:c1], axis=1
                    ),
                ).then_inc(sem, 16)

        @block.vector
        def _(v):
            v.wait_ge(S, 16)
            # Flat element index of feature[r, label[r]] = r*C + label[r].
            # On DVE (native int add) so no GPSIMD ALU library load sits on
            # the critical path.
            v.tensor_tensor(
                out=idx[:],
                in0=lp[:, 0, :],
                in1=lp[:, 1, :],
                op=mybir.AluOpType.add,
            ).then_inc(S, 1)
            last = None
            for i, ((c0, c1), sem) in enumerate(zip(CHUNK_COLS, (SA, SB, SC))):
                v.wait_ge(sem, 16)
                last = v.tensor_reduce(
                    out=res2[0:1, i : i + 1],
                    in_=gat[0:1, c0 * P : c1 * P],
                    axis=mybir.AxisListType.X,
                    op=mybir.AluOpType.add,
                )
            last.then_inc(S, 1)

        # S2 is a write-only counter nobody waits on (walrus requires a
        # completion update on every DMA; actual completion is guaranteed
        # by the SP engine's block-exit Drain), so it needs no end-of-run
        # clear and the critical path skips the out-DMA completion latency.

    nc.finalize()
    _NC_CACHE = nc
    return nc


def _run(feature: np.ndarray, label: np.ndarray, **spmd_kwargs):
    nc = _build_nc()
    feature = np.ascontiguousarray(feature, dtype=np.float32)
    lab32 = np.ascontiguousarray(np.asarray(label).astype(np.int32))
    assert feature.shape == (N, C), feature.shape
    assert lab32.shape == (N,), lab32.shape

    row_off = (np.arange(R, dtype=np.int32) * C).astype(np.int32)
    in_maps = [
        {
            "feature": feature[c * R : (c + 1) * R],
            "labpack": np.stack([lab32[c * R : (c + 1) * R], row_off]),
        }
        for c in range(N_CORES)
    ]
    res = bass_utils.run_bass_kernel_spmd(
        nc, in_maps, core_ids=list(range(N_CORES)), **spmd_kwargs
    )
    partials = np.array(
        [m["out"].reshape(-1) for m in res.results], dtype=np.float32
    )
    total = np.float32(partials.sum(dtype=np.float32))
    loss = np.float32(2.0) - np.float32(2.0) * (total / np.float32(SCALE)) / np.float32(N)
    return np.asarray(loss, dtype=np.float32), res


def kernel(feature: np.ndarray, label: np.ndarray) -> np.ndarray:
    loss, _ = _run(feature, label)
    return loss


# revision 43
# speedup vs baseline: 1.1743x; 1.1743x over previous
"""Center-loss kernel for Trainium2 (Bass/Bacc, raw), 8-core data-parallel.

loss = 2 - 2 * (sum_i feature[i, label[i]] / 64) / 8192

Sharding: batch dim (8192 rows) split evenly across 8 NeuronCores.
Each core gathers its 1024 picked elements straight out of DRAM with
indirect DMAs (no full-matrix read), reduces them to one scalar partial
sum on-device, and the host combines the 8 partials.

Implementation notes (hard-won on this toolchain):
- Raw Bacc instead of TileContext: Tile's sem-init preamble + tail
  barrier butterfly cost ~16us on a ~4us kernel. Everything here runs
  on the GPSIMD (Pool) engine in-order with ONE DMA semaphore.
- The HW indirect DMA consumes ONE index per dest partition and fetches
  dest-free-size contiguous elements, so gathering 1024 scattered
  elements takes 8 DMAs of [128,1] (index tile column by column).
- labpack packs labels + constant row offsets so the index add is a
  single tensor_tensor with one sem wait (HW: 1 sync wait per inst).
"""

import sys

if "/opt/trn_rl_repo" not in sys.path:
    sys.path.insert(0, "/opt/trn_rl_repo")

import numpy as np

import concourse.bacc as bacc
import concourse.bass as bass
from concourse import mybir
from concourse import bass_utils

N = 8192          # batch rows
C = 10000         # num classes (feature columns)
N_CORES = 8
R = N // N_CORES  # rows per core
P = 128           # SBUF partitions
K = R // P        # picked elements per partition
SCALE = 64.0

_NC_CACHE = None


def _build_nc() -> bacc.Bacc:
    global _NC_CACHE
    if _NC_CACHE is not None:
        return _NC_CACHE

    # Bass.__init__ ends with const-tile memsets + an all-engine barrier;
    # nothing here reads the const tiles, so skip the barrier (~0.8us off
    # the first DMA). The block-exit barrier is emitted later, unpatched.
    _orig_barrier = bass.Bass.all_engine_barrier
    bass.Bass.all_engine_barrier = lambda self, **kw: None
    try:
        nc = bacc.Bacc(trn_type="TRN2", monotonic_sem_count=0)
    finally:
        bass.Bass.all_engine_barrier = _orig_barrier
    feat = nc.dram_tensor("feature", [R, C], mybir.dt.float32, kind="ExternalInput")
    # labpack[0] = labels, labpack[1] = arange(R)*C (constant row offsets).
    lab = nc.dram_tensor("labpack", [2, R], mybir.dt.int32, kind="ExternalInput")
    # Two per-chunk partial sums; the host adds them. (Single-element
    # offset reads — e.g. res2[0:1, 1:2] as a TT operand or a [1, 2]
    # X-reduce — are miscompiled to zero/elem0 on this toolchain, so the
    # final scalar combine stays off-device.)
    out = nc.dram_tensor("out", [1, 2], mybir.dt.float32, kind="ExternalOutput")

    # Chunk boundaries in idx columns (128 picks per column). Two equal
    # chunks measured fastest: more chunks pay extra ~1us desc-gen issues
    # on Pool, fewer serialize the whole ring behind one issue.
    CHUNK_COLS = ((0, 4), (4, 8))

    with (
        nc.sbuf_tensor("lp", [P, 2, K], mybir.dt.int32) as lp,
        nc.sbuf_tensor("idx", [P, K], mybir.dt.int32) as idx,
        nc.sbuf_tensor("gat", [1, R], mybir.dt.float32) as gat,
        nc.sbuf_tensor("res2", [1, 2], mybir.dt.float32) as res2,
        nc.sbuf_tensor("warm_idx", [2, 1], mybir.dt.int32) as warm_idx,
        nc.sbuf_tensor("warm_tt", [2, 1], mybir.dt.int32) as warm_tt,
        nc.sbuf_tensor("warm_dst", [1, 2], mybir.dt.float32) as warm_dst,
        nc.semaphore() as S,
        nc.semaphore() as SA,
        nc.semaphore() as SB,
        nc.semaphore() as SC,
        nc.semaphore() as S2,
        nc.Block(no_gpsimd_drain=True) as block,
    ):
        # S: 16 labpack DMA done, 17 idx ready (DVE), 18 reduces done.
        # SA/SB: gather chunk 0/1 done (separate sems — qPoolDynamic chunk
        # completions are NOT ordered). The out DMA only incs S2, a
        # write-only counter nobody waits on (completion is guaranteed by
        # the SP engine's block-exit Drain), so the critical path skips its
        # completion latency and S2 needs no clear. SP clears the waited-on
        # sems once compute is done so re-executions of the loaded NEFF
        # start from zero (NRT only zeroes sems at load).

        @block.sync
        def _(sp):
            # Local row r = p*K + j lives at tile position [p, :, j].
            sp.dma_start(
                out=lp[:], in_=lab[:].rearrange("t (p k) -> p t k", p=P)
            ).then_inc(S, 16)
            sp.wait_ge(S, 18)
            sp.dma_start(out=out[:], in_=res2[:]).then_inc(S2, 16)
            sp.sem_clear(S)
            sp.sem_clear(SA)
            sp.sem_clear(SB)
            sp.sem_clear(SC)

        @block.gpsimd
        def _(g):
            # Warm-up: a 2-descriptor indirect gather at the head of the
            # Pool stream loads the SWDGE desc-gen ucode library while the
            # labpack DMA + index add are still in flight.
            g.memset(warm_idx[:], 0)
            g.indirect_dma_start(
                out=warm_dst[:].rearrange("p (n one) -> p n one", one=1),
                out_offset=None,
                in_=feat[:],
                in_offset=bass.IndirectOffsetOnAxis(ap=warm_idx[:], axis=1),
            ).then_inc(S2, 16)
            g.wait_ge(S, 17)
            # Indirect gathers, chunked so the first chunk's reduce hides
            # under the second chunk's transfer. Walrus generates one
            # descriptor per entry of the dest AP's second-to-last dim, so a
            # [1, H, 1] dest on one partition yields H single-element
            # descriptors; the offset tile is consumed partition-fastest
            # (a permutation of our [p, j] order — irrelevant for the sum).
            for (c0, c1), sem in zip(CHUNK_COLS, (SA, SB)):
                g.indirect_dma_start(
                    out=gat[0:1, c0 * P : c1 * P].rearrange(
                        "p (n one) -> p n one", one=1
                    ),
                    out_offset=None,
                    in_=feat[:],
                    # axis=1 -> coef = prod(shape[2:]) = 1: indices are flat
                    # element offsets into the contiguous [R, C] block.
                    in_offset=bass.IndirectOffsetOnAxis(
                        ap=idx[:, c0:c1], axis=1
                    ),
                ).then_inc(sem, 16)

        @block.vector
        def _(v):
            v.wait_ge(S, 16)
            # Flat element index of feature[r, label[r]] = r*C + label[r].
            # On DVE (native int add) so no GPSIMD ALU library load sits on
            # the critical path.
            v.tensor_tensor(
                out=idx[:],
                in0=lp[:, 0, :],
                in1=lp[:, 1, :],
                op=mybir.AluOpType.add,
            ).then_inc(S, 1)
            last = None
            for i, ((c0, c1), sem) in enumerate(zip(CHUNK_COLS, (SA, SB))):
                v.wait_ge(sem, 16)
                last = v.tensor_reduce(
                    out=res2[0:1, i : i + 1],
                    in_=gat[0:1, c0 * P : c1 * P],
                    axis=mybir.AxisListType.X,
                    op=mybir.AluOpType.add,
                )
            last.then_inc(S, 1)

        # S2 is a write-only counter nobody waits on (walrus requires a
        # completion update on every DMA; actual completion is guaranteed
        # by the SP engine's block-exit Drain), so it needs no end-of-run
        # clear and the critical path skips the out-DMA completion latency.

    nc.finalize()
    _NC_CACHE = nc
    return nc


def _run(feature: np.ndarray, label: np.ndarray, **spmd_kwargs):
    nc = _build_nc()
    feature = np.ascontiguousarray(feature, dtype=np.float32)
    lab32 = np.ascontiguousarray(np.asarray(label).astype(np.int32))
    assert feature.shape == (N, C), feature.shape
    assert lab32.shape == (N,), lab32.shape

    row_off = (np.arange(R, dtype=np.int32) * C).astype(np.int32)
    in_maps = [
        {
            "feature": feature[c * R : (c + 1) * R],
            "labpack": np.stack([lab32[c * R : (c + 1) * R], row_off]),
        }
        for c in range(N_CORES)
    ]
    res = bass_utils.run_bass_kernel_spmd(
        nc, in_maps, core_ids=list(range(N_CORES)), **spmd_kwargs
    )
    partials = np.array(
        [m["out"].reshape(-1) for m in res.results], dtype=np.float32
    )
    total = np.float32(partials.sum(dtype=np.float32))
    loss = np.float32(2.0) - np.float32(2.0) * (total / np.float32(SCALE)) / np.float32(N)
    return np.asarray(loss, dtype=np.float32), res


def kernel(feature: np.ndarray, label: np.ndarray) -> np.ndarray:
    loss, _ = _run(feature, label)
    return loss
